# revision 5
# baseline (speedup 1.0000x reference)
"""BiDirectional LSTM (B=32, T=512, D=H=512, hard_sigmoid gates, output=fwd+bwd sum)
on 8 Trainium2 NeuronCores — v2.

Sharding: core c in 0..7 -> direction d = c//4 (0=fwd, 1=bwd), batch shard s = c%4
(8 samples each). Backward direction realized in data: host feeds bwd cores
time-reversed x; outputs stack in iteration order, so fwd+bwd add at equal indices.

v2 changes vs v1:
  - xz (input projections, all 4 gates) stays resident in SBUF as bf16
    ([128, 16, Tn, 8] = 128KB/partition) -- no DRAM roundtrip, no phase-2 DMA.
  - y history kept bf16 and doubles as the h state read by the recurrence
    matmuls (no h copy per step).
  - hard_sigmoid on DVE via dual-op tensor_scalar (mult/add then min/max);
    ACT only does the two tanh per step.
  - gate order i,f,c,o with o last so the c-chain overlaps o's matmuls and the
    step tail is just zo-add -> 2x tensor_scalar -> y mul.
"""

import numpy as np
import ml_dtypes

B, T, D, H = 32, 512, 512, 512
NCORES = 8
BC = B // 4          # 8 samples per core
KT = D // 128        # 4 k-tiles
MT = (4 * H) // 128  # 16 m-tiles (4 gates x 4 chunks)


def build(nc, Tn=T, repeat=1):
    import concourse.mybir as mybir
    from concourse.tile import TileContext

    f32 = mybir.dt.float32
    bf16 = mybir.dt.bfloat16
    AF = mybir.ActivationFunctionType
    ALU = mybir.AluOpType
    NT = Tn * BC  # GEMM moving free size

    xT = nc.declare_dram_parameter("xT", [KT, 128, NT], bf16, isOutput=False)
    w = nc.declare_dram_parameter("w", [KT, 128, 4 * H], bf16, isOutput=False)
    u = nc.declare_dram_parameter("u", [KT, 128, 4 * H], bf16, isOutput=False)
    bias = nc.declare_dram_parameter("bias", [128, MT], f32, isOutput=False)
    y = nc.declare_dram_parameter("y", [128, Tn, KT, BC], bf16, isOutput=True)

    with TileContext(nc) as tc:
        with (
            tc.tile_pool(name="const", bufs=1) as cpool,
            tc.tile_pool(name="state", bufs=1) as spool,
        ):
            u_sb = [cpool.tile([128, 4 * H], bf16, name=f"u{k}", tag=f"u{k}") for k in range(KT)]
            bias_sb = cpool.tile([128, MT], f32, name="bias", tag="bias")
            for k in range(KT):
                nc.sync.dma_start(out=u_sb[k], in_=u[k])
            nc.sync.dma_start(out=bias_sb, in_=bias[:])

            xz_sb = spool.tile([128, MT, Tn, BC], bf16, name="xz_sb", tag="xz_sb")
            y_hist = spool.tile([128, Tn, KT, BC], bf16, name="y_hist", tag="y_hist")
            c_st = spool.tile([128, KT, BC], f32, name="c_st", tag="c_st")
            h0 = spool.tile([128, KT, BC], bf16, name="h0", tag="h0")
            nc.any.memzero(c_st)
            nc.any.memzero(h0)

            # ---------------- Phase 1: input GEMM (xz = x @ W + b) ----------------
            NCK = min(512, NT)
            NCH = NT // NCK
            with (
                tc.tile_pool(name="wpool", bufs=1) as wpool,
                tc.tile_pool(name="xtp", bufs=2) as xtp,
                tc.tile_pool(name="gpsum", bufs=2, space="PSUM") as gpsum,
            ):
                w_sb = [wpool.tile([128, 4 * H], bf16, name=f"w{k}", tag=f"w{k}") for k in range(KT)]
                for k in range(KT):
                    nc.sync.dma_start(out=w_sb[k], in_=w[k])
                for nci in range(NCH):
                    xtc = xtp.tile([128, KT, NCK], bf16, name="xtc", tag="xtc")
                    for k in range(KT):
                        nc.sync.dma_start(
                            out=xtc[:, k, :], in_=xT[k][:, nci * NCK : (nci + 1) * NCK]
                        )
                    for m in range(MT):
                        ps = gpsum.tile([128, NCK], f32, name="gp", tag="gp")
                        for k in range(KT):
                            nc.tensor.matmul(
                                ps,
                                lhsT=w_sb[k][:, m * 128 : (m + 1) * 128],
                                rhs=xtc[:, k, :],
                                start=(k == 0),
                                stop=(k == KT - 1),
                            )
                        nc.scalar.activation(
                            xz_sb[:, m].rearrange("p t b -> p (t b)")[
                                :, nci * NCK : (nci + 1) * NCK
                            ],
                            ps,
                            AF.Identity,
                            bias=bias_sb[:, m : m + 1],
                            scale=1.0,
                        )

            # ---------------- Phase 2: recurrence ----------------
            # gate layout in u/w columns: [i | f | c | o]
            with (
                tc.tile_pool(name="rpsum", bufs=2, space="PSUM") as rpsum,
                tc.tile_pool(name="ztmp", bufs=2) as zpool,
            ):
                for tt in range(repeat * Tn):
                    t = tt % Tn
                    if t == 0 and tt > 0:
                        nc.any.memzero(c_st)  # repeat>1 timing builds: reset state
                    h_prev = h0 if t == 0 else y_hist[:, t - 1]
                    psg = {
                        g: rpsum.tile([128, KT, BC], f32, name=f"ps{g}", tag=f"ps{g}")
                        for g in range(4)
                    }
                    for g in range(4):
                        for mi in range(4):
                            m = g * 4 + mi
                            for k in range(KT):
                                nc.tensor.matmul(
                                    psg[g][:, mi, :],
                                    lhsT=u_sb[k][:, m * 128 : (m + 1) * 128],
                                    rhs=h_prev[:, k, :],
                                    start=(k == 0),
                                    stop=(k == KT - 1),
                                )
                    # i, f gates: hard_sigmoid fully on DVE
                    sig = {}
                    for g in (0, 1):
                        zt = zpool.tile([128, KT, BC], f32, name=f"z{g}", tag=f"z{g}")
                        nc.vector.tensor_add(zt, psg[g], xz_sb[:, g * 4 : (g + 1) * 4, t, :])
                        st = zpool.tile([128, KT, BC], f32, name=f"s{g}", tag=f"s{g}")
                        nc.vector.tensor_scalar(st, zt, 0.2, 0.5, ALU.mult, ALU.add)
                        rt = zpool.tile([128, KT, BC], f32, name=f"r{g}", tag=f"r{g}")
                        nc.vector.tensor_scalar(rt, st, 1.0, 0.0, ALU.min, ALU.max)
                        sig[g] = rt
                    # t1 = f * c_prev (under c/o matmuls)
                    t1 = zpool.tile([128, KT, BC], f32, name="t1", tag="t1")
                    nc.vector.tensor_mul(t1, sig[1], c_st)
                    # c~ gate: tanh
                    zc = zpool.tile([128, KT, BC], f32, name="zc", tag="zc")
                    nc.vector.tensor_add(zc, psg[2], xz_sb[:, 8:12, t, :])
                    gt = zpool.tile([128, KT, BC], f32, name="gt", tag="gt")
                    nc.scalar.activation(gt, zc, AF.Tanh)
                    t2 = zpool.tile([128, KT, BC], f32, name="t2", tag="t2")
                    nc.vector.tensor_mul(t2, sig[0], gt)
                    nc.vector.tensor_add(c_st, t1, t2)
                    th = zpool.tile([128, KT, BC], f32, name="th", tag="th")
                    nc.scalar.activation(th, c_st, AF.Tanh)
                    # o gate (the only post-last-matmul tail), then h/y
                    zo = zpool.tile([128, KT, BC], f32, name="zo", tag="zo")
                    nc.vector.tensor_add(zo, psg[3], xz_sb[:, 12:16, t, :])
                    so = zpool.tile([128, KT, BC], f32, name="so", tag="so")
                    nc.vector.tensor_scalar(so, zo, 0.2, 0.5, ALU.mult, ALU.add)
                    ro = zpool.tile([128, KT, BC], f32, name="ro", tag="ro")
                    nc.vector.tensor_scalar(ro, so, 1.0, 0.0, ALU.min, ALU.max)
                    nc.vector.tensor_mul(y_hist[:, t], ro, th)

            nc.sync.dma_start(out=y[:], in_=y_hist)
    return nc


def _prep_core_inputs(x, weights, core, Tn=T):
    """weights: dict with all 24 weight arrays (np float32)."""
    d = core // 4
    s = core % 4
    pre = "" if d == 0 else "b"
    gates = ["i", "f", "c", "o"]
    Wc = np.concatenate([weights[f"W{pre}_{g}"] for g in gates], axis=1)
    Uc = np.concatenate([weights[f"U{pre}_{g}"] for g in gates], axis=1)
    bc = np.concatenate([weights[f"b{pre}_{g}"] for g in gates], axis=0)
    xc = x[s * BC : (s + 1) * BC, :Tn]
    if d == 1:
        xc = xc[:, ::-1]
    # [b, t, d] -> [d, t, b] -> [KT, 128, Tn*BC]
    xTc = np.ascontiguousarray(xc.transpose(2, 1, 0)).reshape(KT, 128, Tn * BC)
    return {
        "xT": xTc.astype(ml_dtypes.bfloat16),
        "w": Wc.reshape(KT, 128, 4 * H).astype(ml_dtypes.bfloat16),
        "u": Uc.reshape(KT, 128, 4 * H).astype(ml_dtypes.bfloat16),
        "bias": np.ascontiguousarray(bc.reshape(MT, 128).T).astype(np.float32),
    }


def _gather(results, Tn=T):
    out = np.empty((B, Tn, H), np.float32)
    for s in range(4):
        acc = None
        for d in range(2):
            yc = np.asarray(results[d * 4 + s]["y"], dtype=np.float32)  # [128,Tn,KT,BC]
            part = yc.transpose(3, 1, 2, 0).reshape(BC, Tn, H)
            acc = part if acc is None else acc + part
        out[s * BC : (s + 1) * BC] = acc
    return out


def run(inputs, Tn=T, trace=False):
    import concourse.bacc as bacc
    from concourse.bass_utils import run_bass_kernel_spmd

    x = np.asarray(inputs["x"], np.float32)
    weights = {k: np.asarray(v, np.float32) for k, v in inputs.items() if k != "x"}
    nc = bacc.Bacc("TRN2", target_bir_lowering=False)
    build(nc, Tn)
    nc.compile()
    in_maps = [_prep_core_inputs(x, weights, c, Tn) for c in range(NCORES)]
    res = run_bass_kernel_spmd(nc, in_maps, list(range(NCORES)), trace=trace)
    return _gather(res.results, Tn), res


def kernel(**inputs):
    out, _ = run(inputs)
    return out
